# revision 8
# baseline (speedup 1.0000x reference)
"""ConvergedInhibition forward on 8 Trainium2 NeuronCores.

The reference computes, independently for every (n, h, w) pixel, a
frequency-domain deconvolution along the channel axis C=128:

    out = ifft(fft(x, axis=C) / Fk).real

Division by Fk in frequency space is circular convolution with
g = ifft(1/Fk) (real, since delta-k is real), i.e. a fixed 128x128
circulant matrix M applied to every channel vector:

    out[n, :, h, w] = M @ x[n, :, h, w],   M[c, c'] = g[(c - c') mod C]

M = I + R with ||R||_F/sqrt(C) ~ 0.18, so the forward is a residual
update: out = x + R @ x. The kernel computes the correction R @ x on
device -- the full C*C matmul swept over every pixel -- and the
identity term is folded into the host-side unshard (an elementwise
add against the original fp32 input while gathering core outputs).

That split lets both directions of HBM traffic ride fp8: the rel-err
budget (2e-2) is ~20x looser than what e4m3 round-off contributes
through R (inputs and the correction are attenuated by ||R|| relative
to the output; Monte-Carlo rel err ~9e-3). Weights are stored as
64*R so their e4m3 quantization stays in the normal range, the PSUM
result 64*(R@x) is written back as e4m3, and the host divides by 64
(exact, power of two). HBM traffic per core is 8.4 MB (1 byte/elem
each way) against a ~420 GB/s/core full-duplex ceiling -> ~20 us
data phase, vs 94 us for the original fp32 round trip.

Sharding: data-parallel over batch N=64 -> 8 batches per core, no
cross-core communication. All 8 input row-tiles (128 x 4096 fp8,
512 KB) are resident in SBUF, so every input DMA is enqueued upfront
on the sync engine's HWDGE queue with no dependencies; output DMAs
follow on the same queue, each gated only on its casts. Per tile:
8 matmuls (N=512, one PSUM bank each) land in two 4-bank PSUM tiles,
each drained by a single wide 2048-col fp32->e4m3 cast -- one on DVE,
one on the scalar engine, in parallel on disjoint banks. The filter
preprocessing (length-128 FFT) runs on host in float64.
"""

import numpy as np

import concourse.bass as bass
import concourse.mybir as mybir
from concourse import bacc
from concourse.bass_utils import run_bass_kernel_spmd
from concourse.tile import TileContext

N_CORES = 8
PSUM_CHUNK = 512  # fp32 elements per PSUM bank
W_SCALE = 64.0  # weights stored as W_SCALE*R; host divides the result back


def _residual_circulant(filt: np.ndarray, C: int) -> np.ndarray:
    """Build lhsT (K x M layout) for the correction operator R = M - I.

    out[m] = sum_k M[m, k] x[k] with M[m, k] = g[(m - k) mod C], and the
    tensor engine computes lhsT.T @ rhs, so lhsT[k, m] = g[(m - k) mod C].
    """
    scope = filt.shape[-1]
    pad_left = (C - scope) // 2
    k = np.zeros(C, dtype=np.float64)
    k[pad_left : pad_left + scope] = filt.reshape(-1).astype(np.float64)
    k = np.roll(k, C // 2 + 1)
    delta = np.zeros(C, dtype=np.float64)
    delta[0] = 1.0
    g = np.fft.ifft(1.0 / np.fft.fft(delta - k)).real
    j = np.arange(C)
    return g[(j[None, :] - j[:, None]) % C] - np.eye(C)


def build_nc(b_per_core: int, C: int, P: int) -> bacc.Bacc:
    io_dt = mybir.dt.float8e4
    quarter = P // 4  # 1024: one 2-bank PSUM tile / one cast op
    nc = bacc.Bacc("TRN2", target_bir_lowering=False, debug=False)
    x = nc.dram_tensor("x", [b_per_core, C, P], io_dt, kind="ExternalInput")
    w = nc.dram_tensor("w", [C, C], io_dt, kind="ExternalInput")
    y = nc.dram_tensor("y", [b_per_core, C, P], io_dt, kind="ExternalOutput")

    last = b_per_core - 1
    with TileContext(nc) as tc:
        with (
            tc.tile_pool(name="wp", bufs=1) as wp,
            tc.tile_pool(name="xq", bufs=4) as xqp,
            tc.tile_pool(name="xp", bufs=b_per_core - 1) as xp,
            tc.tile_pool(name="yp", bufs=b_per_core - 1) as yp,
            tc.tile_pool(name="yh", bufs=2) as yhp,
            tc.tile_pool(name="pp", bufs=4, space="PSUM") as pp,
        ):
            # Everything fits in SBUF at 1 byte/elem (8 MB total), so all
            # input loads are enqueued upfront with no pool recycling: the
            # sync HWDGE queue streams them back to back while compute and
            # output DMAs trail behind. Tile 0 is loaded as four quarter
            # tiles so its first matmuls fire ~1.3 us earlier.
            wt = wp.tile([C, C], io_dt)
            nc.sync.dma_start(wt[:], w[:, :])
            x0q = []
            for q in range(4):
                xt = xqp.tile([C, quarter], io_dt, tag="xq")
                nc.sync.dma_start(xt[:], x[0, :, bass.ds(q * quarter, quarter)])
                x0q.append(xt)
            xts = [None]
            for b in range(1, b_per_core):
                xt = xp.tile([C, P], io_dt, tag="x")
                nc.sync.dma_start(xt[:], x[b])
                xts.append(xt)

            for b in range(b_per_core):
                # Four 2-bank PSUM tiles per row-tile (bufs=4 = all 8 banks)
                # keep matmuls for the next quarter running while the two
                # copy engines drain earlier quarters: DVE casts quarters
                # 0-1, the scalar engine quarters 2-3, concurrently on
                # disjoint banks.
                if b == last:
                    yh0 = yhp.tile([C, 2 * quarter], io_dt, tag="yh")
                    yh1 = yhp.tile([C, 2 * quarter], io_dt, tag="yh")
                    yhalves = [yh0, yh1]
                else:
                    yt = yp.tile([C, P], io_dt, tag="y")
                for q in range(4):
                    pt = pp.tile([C, quarter], mybir.dt.float32)
                    if b == 0 and q == 0:
                        # Warm the PE HAM out of its cold half-clock while
                        # tile 0 is still in flight: a few matmuls against
                        # the (already landed) weight tile into this PSUM
                        # tile, overwritten by the real matmuls below.
                        for _ in range(5):
                            nc.tensor.matmul(
                                pt[:, : C], wt[:], wt[:], start=True, stop=True
                            )
                    for j in range(quarter // PSUM_CHUNK):
                        cols = bass.ds(j * PSUM_CHUNK, PSUM_CHUNK)
                        if b == 0:
                            rhs = x0q[q][:, cols]
                        else:
                            rhs = xts[b][:, bass.ds(q * quarter + j * PSUM_CHUNK, PSUM_CHUNK)]
                        nc.tensor.matmul(
                            pt[:, cols], wt[:], rhs, start=True, stop=True
                        )
                    if b == last:
                        dst = yhalves[q // 2][:, bass.ds(q % 2 * quarter, quarter)]
                    else:
                        dst = yt[:, bass.ds(q * quarter, quarter)]
                    if q < 2:
                        nc.vector.tensor_copy(dst, pt[:])
                    else:
                        nc.scalar.copy(dst, pt[:])
                # Outputs ride the same sync HWDGE queue: every input is
                # already enqueued ahead of them, so the engine-level wait
                # on the casts here blocks nothing. The last tile goes out
                # as two halves so the final DMA chases the last cast by
                # half a tile instead of a full one.
                if b == last:
                    for h in range(2):
                        nc.sync.dma_start(
                            y[b, :, bass.ds(h * 2 * quarter, 2 * quarter)], yhalves[h][:]
                        )
                else:
                    nc.sync.dma_start(y[b], yt[:])
    nc.compile()
    return nc


_NC_CACHE: dict = {}


def _run(activations, inhibition_filter, **spmd_kwargs):
    act = np.asarray(activations, dtype=np.float32)
    filt = np.asarray(inhibition_filter, dtype=np.float32)
    B, C, H, W = act.shape
    P = H * W
    assert B % N_CORES == 0
    b_per_core = B // N_CORES

    f8 = mybir.dt.np(mybir.dt.float8e4)
    lhsT = (_residual_circulant(filt, C) * W_SCALE).astype(f8)
    key = (b_per_core, C, P)
    nc = _NC_CACHE.get(key)
    if nc is None:
        nc = _NC_CACHE[key] = build_nc(b_per_core, C, P)

    xs = act.reshape(N_CORES, b_per_core, C, P).astype(f8)
    in_maps = [{"x": xs[i], "w": lhsT} for i in range(N_CORES)]
    res = run_bass_kernel_spmd(nc, in_maps, core_ids=list(range(N_CORES)), **spmd_kwargs)
    corr = np.stack([res.results[i]["y"] for i in range(N_CORES)], axis=0)
    out = act + corr.reshape(B, C, H, W).astype(np.float32) * np.float32(1.0 / W_SCALE)
    return out, res


def kernel(activations: np.ndarray, inhibition_filter: np.ndarray) -> np.ndarray:
    out, _ = _run(activations, inhibition_filter)
    return out
